# revision 6
# baseline (speedup 1.0000x reference)
"""Trainium2 Bass kernel for nn_CPCircuitLayer (embedding_lookup).

Math: A = X @ W_seq^T  [S,R];  Bm = X^T @ W_hid^T  [H,R]
      out[b, n] = dot(A[b, idx_s[n]], Bm[b, idx_h[n]]),  out -> [B, S, H]

Sharding (8 cores, no collectives): core c handles batch b = c//4 and the
quarter q = c%4 of the N = S*H index list (J = N/4 indices, both factor
tables computed redundantly per batch group from the full X[b]).

Per-core device pipeline:
  1. Load X[b] (bf16) + transposed copy via HWDGE transpose-DMA.
  2. PE matmuls (bf16 in, f32 psum): A rows and Bm rows -> DRAM tables
     [1024, 64] f32 (only first 32 cols valid; 256B row stride).
  3. dma_gather (SWDGE ucode): for each round of 1024 indices, gather the
     256B rows of A and Bm picked by idx_s / idx_h.
  4. DVE: prod = ga * gb over the valid 32 lanes, segmented reduce -> out.
"""

import os
import numpy as np
import ml_dtypes
from contextlib import ExitStack

import concourse.bass as bass
import concourse.mybir as mybir
import concourse.tile as tile
from concourse import bacc
from concourse.bass_utils import run_bass_kernel_spmd

B, S, H, R = 2, 1024, 1024, 32
N = S * H
NCORES = 8
J = N // 4            # outputs per core (one batch, quarter of N) = 262144
NI = 1024             # indices per dma_gather (ring-capacity limit ~1024)
ROUNDS = J // NI      # 256
GRP = 4               # rounds per DVE mul/reduce group
CHUNK = NI // 128     # dst chunks per round = 8
PAD = 64              # table row f32 elems (256B rows)

F32 = mybir.dt.float32
BF16 = mybir.dt.bfloat16
I16 = mybir.dt.int16

_nc_cache = None


def _build():
    nc = bacc.Bacc()
    x = nc.declare_dram_parameter("x", [S, H], BF16, False)
    wseq_t = nc.declare_dram_parameter("wseq_t", [H, R], BF16, False)
    whid_t = nc.declare_dram_parameter("whid_t", [S, R], BF16, False)
    idx_s = nc.declare_dram_parameter("idx_s", [16, J // 16], I16, False)
    idx_h = nc.declare_dram_parameter("idx_h", [16, J // 16], I16, False)
    out = nc.declare_dram_parameter("out", [128, ROUNDS * CHUNK], F32, True)
    a_dram = nc.dram_tensor("a_rows", [S, PAD], F32)
    b_dram = nc.dram_tensor("b_rows", [H, PAD], F32)

    with tile.TileContext(nc) as tc, ExitStack() as ctx:
        base = ctx.enter_context(tc.tile_pool(name="base", bufs=1))
        psum = ctx.enter_context(tc.tile_pool(name="psum", bufs=2, space="PSUM"))
        stage = ctx.enter_context(tc.tile_pool(name="stage", bufs=2))
        gap = ctx.enter_context(tc.tile_pool(name="gap", bufs=2))
        gbp = ctx.enter_context(tc.tile_pool(name="gbp", bufs=2))
        prodp = ctx.enter_context(tc.tile_pool(name="prodp", bufs=2))

        # --- loads -------------------------------------------------------
        x_sb = base.tile([128, 8, H], BF16)       # X[s,h]: p=s%128, k=s//128
        xt_sb = base.tile([128, 8, S], BF16)      # X^T[h,s]: p=h%128, k=h//128
        ws_sb = base.tile([128, 8, R], BF16)      # W_seq^T rows (h-major)
        wh_sb = base.tile([128, 8, R], BF16)      # W_hid^T rows (s-major)
        isb_s = base.tile([128, J // 16], I16)
        isb_h = base.tile([128, J // 16], I16)
        out_sb = base.tile([128, ROUNDS * CHUNK], F32)

        nc.sync.dma_start(
            out=x_sb[:],
            in_=bass.AP(tensor=x[:].tensor, offset=0,
                        ap=[[H, 128], [128 * H, 8], [1, H]]),
        )
        for k in range(8):
            nc.sync.dma_start_transpose(
                out=xt_sb[:, k, :], in_=x[:, 128 * k:128 * (k + 1)]
            )
        nc.sync.dma_start(
            out=ws_sb[:],
            in_=bass.AP(tensor=wseq_t[:].tensor, offset=0,
                        ap=[[R, 128], [128 * R, 8], [1, R]]),
        )
        nc.sync.dma_start(
            out=wh_sb[:],
            in_=bass.AP(tensor=whid_t[:].tensor, offset=0,
                        ap=[[R, 128], [128 * R, 8], [1, R]]),
        )
        # idx: replicate [16, J/16] across the 8 partition groups
        nc.sync.dma_start(
            out=isb_s[:],
            in_=bass.AP(tensor=idx_s[:].tensor, offset=0,
                        ap=[[0, 8], [J // 16, 16], [1, J // 16]]),
        )
        nc.sync.dma_start(
            out=isb_h[:],
            in_=bass.AP(tensor=idx_h[:].tensor, offset=0,
                        ap=[[0, 8], [J // 16, 16], [1, J // 16]]),
        )

        # --- factor tables ----------------------------------------------
        # A rows: A[s, r] = sum_h X[s, h] Wseq^T[h, r]
        #   lhsT = X^T tile [128h, 128s], rhs = Wseq^T tile [128h, 32]
        table_writes = []
        for m in range(8):
            pa = psum.tile([128, R], F32, tag="pa")
            for k in range(8):
                nc.tensor.matmul(
                    out=pa[:],
                    lhsT=xt_sb[:, k, 128 * m:128 * (m + 1)],
                    rhs=ws_sb[:, k, :],
                    start=(k == 0), stop=(k == 7),
                )
            sa = stage.tile([128, PAD], F32, tag="sa")
            nc.vector.memset(sa[:, R:PAD], 0.0)
            nc.vector.tensor_copy(out=sa[:, 0:R], in_=pa[:])
            w = nc.gpsimd.dma_start(
                out=a_dram[128 * m:128 * (m + 1), :], in_=sa[:]
            )
            table_writes.append(w)
        # B rows: Bm[h, r] = sum_s X[s, h] Whid^T[s, r]
        #   lhsT = X tile [128s, 128h], rhs = Whid^T tile [128s, 32]
        for m in range(8):
            pb = psum.tile([128, R], F32, tag="pa")
            for k in range(8):
                nc.tensor.matmul(
                    out=pb[:],
                    lhsT=x_sb[:, k, 128 * m:128 * (m + 1)],
                    rhs=wh_sb[:, k, :],
                    start=(k == 0), stop=(k == 7),
                )
            sb = stage.tile([128, PAD], F32, tag="sa")
            nc.vector.memset(sb[:, R:PAD], 0.0)
            nc.vector.tensor_copy(out=sb[:, 0:R], in_=pb[:])
            w = nc.gpsimd.dma_start(
                out=b_dram[128 * m:128 * (m + 1), :], in_=sb[:]
            )
            table_writes.append(w)

        # --- gather + reduce --------------------------------------------
        from concourse.tile_rust import add_dep_helper

        for g in range(ROUNDS // GRP):
            ga = gap.tile([128, GRP * CHUNK, PAD], F32, tag="ga")
            gb = gbp.tile([128, GRP * CHUNK, PAD], F32, tag="gb")
            for rr in range(GRP):
                r = g * GRP + rr
                isl = slice(r * (NI // 16), (r + 1) * (NI // 16))
                dsl = slice(rr * CHUNK, (rr + 1) * CHUNK)
                g1 = nc.gpsimd.dma_gather(
                    ga[:, dsl, :], a_dram[:], isb_s[:, isl], NI, NI, PAD
                )
                g2 = nc.gpsimd.dma_gather(
                    gb[:, dsl, :], b_dram[:], isb_h[:, isl], NI, NI, PAD
                )
            prod = prodp.tile([128, GRP * CHUNK, R], F32, tag="prod")
            nc.vector.tensor_mul(prod[:], ga[:, :, 0:R], gb[:, :, 0:R])
            nc.vector.tensor_reduce(
                out=out_sb[:, g * GRP * CHUNK:(g + 1) * GRP * CHUNK],
                in_=prod[:],
                axis=mybir.AxisListType.X,
                op=mybir.AluOpType.add,
            )
        nc.sync.dma_start(out=out[:], in_=out_sb[:])
    nc.compile()
    return nc


def _get_nc():
    global _nc_cache
    if _nc_cache is None:
        _nc_cache = _build()
    return _nc_cache


def _wrap_idx(v: np.ndarray) -> np.ndarray:
    """[J] int -> [16, J/16] int16 in dma_gather wrapped layout.

    Round r (NI indices) occupies columns [r*NI/16, (r+1)*NI/16); within a
    round index j sits at [j % 16, j // 16].
    """
    w = v.astype(np.int16).reshape(ROUNDS, NI // 16, 16)
    w = w.transpose(2, 0, 1).reshape(16, J // 16)
    return np.ascontiguousarray(w)


def kernel(hidden_states, W_seq, W_hid, all_indices):
    hidden_states = np.asarray(hidden_states)
    W_seq = np.asarray(W_seq)
    W_hid = np.asarray(W_hid)
    all_indices = np.asarray(all_indices)

    nc = _get_nc()

    x_bf = [np.ascontiguousarray(hidden_states[b].astype(ml_dtypes.bfloat16))
            for b in range(B)]
    ws_t = np.ascontiguousarray(W_seq.T.astype(ml_dtypes.bfloat16))
    wh_t = np.ascontiguousarray(W_hid.T.astype(ml_dtypes.bfloat16))

    idx_pairs = []
    for q in range(4):
        seg = all_indices[q * J:(q + 1) * J]
        idx_pairs.append((_wrap_idx(seg[:, 0]), _wrap_idx(seg[:, 1])))

    in_maps = []
    for c in range(NCORES):
        b, q = c // 4, c % 4
        in_maps.append({
            "x": x_bf[b],
            "wseq_t": ws_t,
            "whid_t": wh_t,
            "idx_s": idx_pairs[q][0],
            "idx_h": idx_pairs[q][1],
        })

    res = run_bass_kernel_spmd(nc, in_maps, list(range(NCORES)))

    out = np.empty((B, N), dtype=np.float32)
    for c in range(NCORES):
        b, q = c // 4, c % 4
        o = res.results[c]["out"].reshape(128, ROUNDS, CHUNK)
        # out_sb[p, r, ch] holds output j = r*NI + ch*128 + p
        out[b, q * J:(q + 1) * J] = o.transpose(1, 2, 0).reshape(J)
    return out.reshape(B, S, H)


# revision 8
# speedup vs baseline: 8.8833x; 8.8833x over previous
"""Trainium2 Bass kernel for nn_CPCircuitLayer (embedding_lookup).

Math: A = X @ W_seq^T  [S,R];  Bm = X^T @ W_hid^T  [H,R]
      out[b, n] = dot(A[b, idx_s[n]], Bm[b, idx_h[n]]),  out -> [B, S, H]

Sharding (8 cores, no collectives): core c handles batch b = c//4 and the
quarter q = c%4 of the N = S*H index list (J = N/4 indices, both factor
tables computed redundantly per batch group from the full X[b]).

Per-core device pipeline:
  1. Load X[b] (bf16) + transposed copy via HWDGE transpose-DMA.
  2. PE matmuls (bf16 in, f32 psum): A rows and Bm rows -> DRAM tables
     [1024, 64] f32 (only first 32 cols valid; 256B row stride).
  3. dma_gather (SWDGE ucode): for each round of 1024 indices, gather the
     256B rows of A and Bm picked by idx_s / idx_h.
  4. DVE: prod = ga * gb over the valid 32 lanes, segmented reduce -> out.
"""

import os
import numpy as np
import ml_dtypes
from contextlib import ExitStack

import concourse.bass as bass
import concourse.mybir as mybir
import concourse.tile as tile
from concourse import bacc
from concourse.bass_utils import run_bass_kernel_spmd

B, S, H, R = 2, 1024, 1024, 32
N = S * H
NCORES = 8
J = N // 4            # outputs per core (one batch, quarter of N) = 262144
NI = 1024             # indices per dma_gather (ring-capacity limit ~1024)
ROUNDS = J // NI      # 256
GRP = 4               # rounds per DVE mul/reduce group
CHUNK = NI // 128     # dst chunks per round = 8
PAD = 64              # table row f32 elems (256B rows)

F32 = mybir.dt.float32
BF16 = mybir.dt.bfloat16
I16 = mybir.dt.int16

_nc_cache = None


def _build(reps: int = 1):
    nc = bacc.Bacc()
    x = nc.declare_dram_parameter("x", [S, H], BF16, False)
    wseq_t = nc.declare_dram_parameter("wseq_t", [H, R], BF16, False)
    whid_t = nc.declare_dram_parameter("whid_t", [S, R], BF16, False)
    idx_s = nc.declare_dram_parameter("idx_s", [16, J // 16], I16, False)
    idx_h = nc.declare_dram_parameter("idx_h", [16, J // 16], I16, False)
    out = nc.declare_dram_parameter("out", [128, ROUNDS * CHUNK], F32, True)
    a_dram = nc.dram_tensor("a_rows", [S, PAD], F32)
    b_dram = nc.dram_tensor("b_rows", [H, PAD], F32)

    with tile.TileContext(nc) as tc, ExitStack() as ctx:
        base = ctx.enter_context(tc.tile_pool(name="base", bufs=1))
        psum = ctx.enter_context(tc.tile_pool(name="psum", bufs=2, space="PSUM"))
        stage = ctx.enter_context(tc.tile_pool(name="stage", bufs=2))
        gap = ctx.enter_context(tc.tile_pool(name="gap", bufs=2))
        gbp = ctx.enter_context(tc.tile_pool(name="gbp", bufs=2))
        prodp = ctx.enter_context(tc.tile_pool(name="prodp", bufs=2))

        # --- loads -------------------------------------------------------
        x_sb = base.tile([128, 8, H], BF16)       # X[s,h]: p=s%128, k=s//128
        xt_sb = base.tile([128, 8, S], BF16)      # X^T[h,s]: p=h%128, k=h//128
        ws_sb = base.tile([128, 8, R], BF16)      # W_seq^T rows (h-major)
        wh_sb = base.tile([128, 8, R], BF16)      # W_hid^T rows (s-major)
        isb_s = base.tile([128, J // 16], I16)
        isb_h = base.tile([128, J // 16], I16)
        out_sb = base.tile([128, ROUNDS * CHUNK], F32)

        nc.sync.dma_start(
            out=x_sb[:],
            in_=bass.AP(tensor=x[:].tensor, offset=0,
                        ap=[[H, 128], [128 * H, 8], [1, H]]),
        )
        for k in range(8):
            nc.sync.dma_start_transpose(
                out=xt_sb[:, k, :], in_=x[:, 128 * k:128 * (k + 1)]
            )
        nc.sync.dma_start(
            out=ws_sb[:],
            in_=bass.AP(tensor=wseq_t[:].tensor, offset=0,
                        ap=[[R, 128], [128 * R, 8], [1, R]]),
        )
        nc.sync.dma_start(
            out=wh_sb[:],
            in_=bass.AP(tensor=whid_t[:].tensor, offset=0,
                        ap=[[R, 128], [128 * R, 8], [1, R]]),
        )
        # idx: replicate [16, J/16] across the 8 partition groups
        nc.sync.dma_start(
            out=isb_s[:],
            in_=bass.AP(tensor=idx_s[:].tensor, offset=0,
                        ap=[[0, 8], [J // 16, 16], [1, J // 16]]),
        )
        nc.sync.dma_start(
            out=isb_h[:],
            in_=bass.AP(tensor=idx_h[:].tensor, offset=0,
                        ap=[[0, 8], [J // 16, 16], [1, J // 16]]),
        )

        # --- body (repeatable for HW-time measurement) -------------------
        for _rep in range(reps):
            _body(nc, tc, base, psum, stage, gap, gbp, prodp,
                  x_sb, xt_sb, ws_sb, wh_sb, isb_s, isb_h, out_sb,
                  a_dram, b_dram, out)
    nc.compile()
    return nc


def _body(nc, tc, base, psum, stage, gap, gbp, prodp,
          x_sb, xt_sb, ws_sb, wh_sb, isb_s, isb_h, out_sb,
          a_dram, b_dram, out):
    if True:
        # A rows: A[s, r] = sum_h X[s, h] Wseq^T[h, r]
        #   lhsT = X^T tile [128h, 128s], rhs = Wseq^T tile [128h, 32]
        table_writes = []
        for m in range(8):
            pa = psum.tile([128, R], F32, tag="pa")
            for k in range(8):
                nc.tensor.matmul(
                    out=pa[:],
                    lhsT=xt_sb[:, k, 128 * m:128 * (m + 1)],
                    rhs=ws_sb[:, k, :],
                    start=(k == 0), stop=(k == 7),
                )
            sa = stage.tile([128, PAD], F32, tag="sa")
            nc.vector.memset(sa[:, R:PAD], 0.0)
            nc.vector.tensor_copy(out=sa[:, 0:R], in_=pa[:])
            w = nc.gpsimd.dma_start(
                out=a_dram[128 * m:128 * (m + 1), :], in_=sa[:]
            )
            table_writes.append(w)
        # B rows: Bm[h, r] = sum_s X[s, h] Whid^T[s, r]
        #   lhsT = X tile [128s, 128h], rhs = Whid^T tile [128s, 32]
        for m in range(8):
            pb = psum.tile([128, R], F32, tag="pa")
            for k in range(8):
                nc.tensor.matmul(
                    out=pb[:],
                    lhsT=x_sb[:, k, 128 * m:128 * (m + 1)],
                    rhs=wh_sb[:, k, :],
                    start=(k == 0), stop=(k == 7),
                )
            sb = stage.tile([128, PAD], F32, tag="sa")
            nc.vector.memset(sb[:, R:PAD], 0.0)
            nc.vector.tensor_copy(out=sb[:, 0:R], in_=pb[:])
            w = nc.gpsimd.dma_start(
                out=b_dram[128 * m:128 * (m + 1), :], in_=sb[:]
            )
            table_writes.append(w)

        # --- gather + reduce --------------------------------------------
        from concourse.tile_rust import add_dep_helper

        for g in range(ROUNDS // GRP):
            ga = gap.tile([128, GRP * CHUNK, PAD], F32, tag="ga")
            gb = gbp.tile([128, GRP * CHUNK, PAD], F32, tag="gb")
            for rr in range(GRP):
                r = g * GRP + rr
                isl = slice(r * (NI // 16), (r + 1) * (NI // 16))
                dsl = slice(rr * CHUNK, (rr + 1) * CHUNK)
                g1 = nc.gpsimd.dma_gather(
                    ga[:, dsl, :], a_dram[:], isb_s[:, isl], NI, NI, PAD
                )
                g2 = nc.gpsimd.dma_gather(
                    gb[:, dsl, :], b_dram[:], isb_h[:, isl], NI, NI, PAD
                )
            prod = prodp.tile([128, GRP * CHUNK, R], F32, tag="prod")
            nc.vector.tensor_mul(prod[:], ga[:, :, 0:R], gb[:, :, 0:R])
            nc.vector.tensor_reduce(
                out=out_sb[:, g * GRP * CHUNK:(g + 1) * GRP * CHUNK],
                in_=prod[:],
                axis=mybir.AxisListType.X,
                op=mybir.AluOpType.add,
            )
        nc.sync.dma_start(out=out[:], in_=out_sb[:])


_nc_cache_by_reps = {}


def _get_nc(reps: int = 1):
    nc = _nc_cache_by_reps.get(reps)
    if nc is None:
        nc = _nc_cache_by_reps[reps] = _build(reps)
    return nc


def prepare_in_maps(hidden_states, W_seq, W_hid, all_indices):
    x_bf = [np.ascontiguousarray(hidden_states[b].astype(ml_dtypes.bfloat16))
            for b in range(B)]
    ws_t = np.ascontiguousarray(W_seq.T.astype(ml_dtypes.bfloat16))
    wh_t = np.ascontiguousarray(W_hid.T.astype(ml_dtypes.bfloat16))
    idx_pairs = []
    for q in range(4):
        seg = all_indices[q * J:(q + 1) * J]
        idx_pairs.append((_wrap_idx(seg[:, 0]), _wrap_idx(seg[:, 1])))
    in_maps = []
    for c in range(NCORES):
        b, q = c // 4, c % 4
        in_maps.append({
            "x": x_bf[b],
            "wseq_t": ws_t,
            "whid_t": wh_t,
            "idx_s": idx_pairs[q][0],
            "idx_h": idx_pairs[q][1],
        })
    return in_maps


def _wrap_idx(v: np.ndarray) -> np.ndarray:
    """[J] int -> [16, J/16] int16 in dma_gather wrapped layout.

    Round r (NI indices) occupies columns [r*NI/16, (r+1)*NI/16); within a
    round index j sits at [j % 16, j // 16].
    """
    w = v.astype(np.int16).reshape(ROUNDS, NI // 16, 16)
    w = w.transpose(2, 0, 1).reshape(16, J // 16)
    return np.ascontiguousarray(w)


def kernel(hidden_states, W_seq, W_hid, all_indices):
    hidden_states = np.asarray(hidden_states)
    W_seq = np.asarray(W_seq)
    W_hid = np.asarray(W_hid)
    all_indices = np.asarray(all_indices)

    nc = _get_nc()
    in_maps = prepare_in_maps(hidden_states, W_seq, W_hid, all_indices)
    res = run_bass_kernel_spmd(nc, in_maps, list(range(NCORES)))

    out = np.empty((B, N), dtype=np.float32)
    for c in range(NCORES):
        b, q = c // 4, c % 4
        o = res.results[c]["out"].reshape(128, ROUNDS, CHUNK)
        # out_sb[p, r, ch] holds output j = r*NI + ch*128 + p
        out[b, q * J:(q + 1) * J] = o.transpose(1, 2, 0).reshape(J)
    return out.reshape(B, S, H)


# revision 9
# speedup vs baseline: 11.5534x; 1.3006x over previous
"""Trainium2 Bass kernel for nn_CPCircuitLayer (embedding_lookup).

Math: A = X @ W_seq^T  [S,R];  Bm = X^T @ W_hid^T  [H,R]
      out[b, n] = dot(A[b, idx_s[n]], Bm[b, idx_h[n]]),  out -> [B, S, H]

Sharding (8 cores, no collectives): core c handles batch b = c//4 and the
quarter q = c%4 of the N = S*H index list (J = N/4 indices, both factor
tables computed redundantly per batch group from the full X[b]).

Per-core device pipeline:
  1. Load X[b] (bf16) + transposed copy via HWDGE transpose-DMA.
  2. PE matmuls (bf16 in, f32 psum): A rows and Bm rows -> DRAM tables
     [1024, 64] f32 (only first 32 cols valid; 256B row stride).
  3. dma_gather (SWDGE ucode): for each round of 1024 indices, gather the
     256B rows of A and Bm picked by idx_s / idx_h.
  4. DVE: prod = ga * gb over the valid 32 lanes, segmented reduce -> out.
"""

import os
import numpy as np
import ml_dtypes
from contextlib import ExitStack

import concourse.bass as bass
import concourse.mybir as mybir
import concourse.tile as tile
from concourse import bacc
from concourse.bass_utils import run_bass_kernel_spmd

B, S, H, R = 2, 1024, 1024, 32
N = S * H
NCORES = 8
J = N // 4            # outputs per core (one batch, quarter of N) = 262144
NI = 1024             # indices per dma_gather (ring-capacity limit ~1024)
ROUNDS = J // NI      # 256
GRP = 8               # rounds per DVE mul/reduce group
CHUNK = NI // 128     # dst chunks per round = 8
PAD = 64              # table row f32 elems (256B rows)

F32 = mybir.dt.float32
BF16 = mybir.dt.bfloat16
I16 = mybir.dt.int16

_nc_cache = None


def _build(reps: int = 1):
    nc = bacc.Bacc(num_swdge_queues=4)
    x = nc.declare_dram_parameter("x", [S, H], BF16, False)
    wseq_t = nc.declare_dram_parameter("wseq_t", [H, R], BF16, False)
    whid_t = nc.declare_dram_parameter("whid_t", [S, R], BF16, False)
    idx_s = nc.declare_dram_parameter("idx_s", [16, J // 16], I16, False)
    idx_h = nc.declare_dram_parameter("idx_h", [16, J // 16], I16, False)
    out = nc.declare_dram_parameter("out", [128, ROUNDS * CHUNK], F32, True)
    a_dram = nc.dram_tensor("a_rows", [S, PAD], F32)
    b_dram = nc.dram_tensor("b_rows", [H, PAD], F32)

    with tile.TileContext(nc) as tc, ExitStack() as ctx:
        base = ctx.enter_context(tc.tile_pool(name="base", bufs=1))
        psum = ctx.enter_context(tc.tile_pool(name="psum", bufs=2, space="PSUM"))
        stage = ctx.enter_context(tc.tile_pool(name="stage", bufs=2))
        gap = ctx.enter_context(tc.tile_pool(name="gap", bufs=2))
        gbp = ctx.enter_context(tc.tile_pool(name="gbp", bufs=2))
        prodp = ctx.enter_context(tc.tile_pool(name="prodp", bufs=2))

        # --- loads -------------------------------------------------------
        x_sb = base.tile([128, 8, H], BF16)       # X[s,h]: p=s%128, k=s//128
        xt_sb = base.tile([128, 8, S], BF16)      # X^T[h,s]: p=h%128, k=h//128
        ws_sb = base.tile([128, 8, R], BF16)      # W_seq^T rows (h-major)
        wh_sb = base.tile([128, 8, R], BF16)      # W_hid^T rows (s-major)
        isb_s = base.tile([128, J // 16], I16)
        isb_h = base.tile([128, J // 16], I16)
        out_sb = base.tile([128, ROUNDS * CHUNK], F32)

        nc.sync.dma_start(
            out=x_sb[:],
            in_=bass.AP(tensor=x[:].tensor, offset=0,
                        ap=[[H, 128], [128 * H, 8], [1, H]]),
        )
        for k in range(8):
            nc.sync.dma_start_transpose(
                out=xt_sb[:, k, :], in_=x[:, 128 * k:128 * (k + 1)]
            )
        nc.sync.dma_start(
            out=ws_sb[:],
            in_=bass.AP(tensor=wseq_t[:].tensor, offset=0,
                        ap=[[R, 128], [128 * R, 8], [1, R]]),
        )
        nc.sync.dma_start(
            out=wh_sb[:],
            in_=bass.AP(tensor=whid_t[:].tensor, offset=0,
                        ap=[[R, 128], [128 * R, 8], [1, R]]),
        )
        # idx: replicate [16, J/16] across the 8 partition groups
        nc.sync.dma_start(
            out=isb_s[:],
            in_=bass.AP(tensor=idx_s[:].tensor, offset=0,
                        ap=[[0, 8], [J // 16, 16], [1, J // 16]]),
        )
        nc.sync.dma_start(
            out=isb_h[:],
            in_=bass.AP(tensor=idx_h[:].tensor, offset=0,
                        ap=[[0, 8], [J // 16, 16], [1, J // 16]]),
        )

        # --- body (repeatable for HW-time measurement) -------------------
        for _rep in range(reps):
            _body(nc, tc, base, psum, stage, gap, gbp, prodp,
                  x_sb, xt_sb, ws_sb, wh_sb, isb_s, isb_h, out_sb,
                  a_dram, b_dram, out)
    nc.compile()
    return nc


def _body(nc, tc, base, psum, stage, gap, gbp, prodp,
          x_sb, xt_sb, ws_sb, wh_sb, isb_s, isb_h, out_sb,
          a_dram, b_dram, out):
    if True:
        # A rows: A[s, r] = sum_h X[s, h] Wseq^T[h, r]
        #   lhsT = X^T tile [128h, 128s], rhs = Wseq^T tile [128h, 32]
        table_writes = []
        for m in range(8):
            pa = psum.tile([128, R], F32, tag="pa")
            for k in range(8):
                nc.tensor.matmul(
                    out=pa[:],
                    lhsT=xt_sb[:, k, 128 * m:128 * (m + 1)],
                    rhs=ws_sb[:, k, :],
                    start=(k == 0), stop=(k == 7),
                )
            sa = stage.tile([128, PAD], F32, tag="sa")
            nc.vector.memset(sa[:, R:PAD], 0.0)
            nc.vector.tensor_copy(out=sa[:, 0:R], in_=pa[:])
            w = nc.gpsimd.dma_start(
                out=a_dram[128 * m:128 * (m + 1), :], in_=sa[:]
            )
            table_writes.append(w)
        # B rows: Bm[h, r] = sum_s X[s, h] Whid^T[s, r]
        #   lhsT = X tile [128s, 128h], rhs = Whid^T tile [128s, 32]
        for m in range(8):
            pb = psum.tile([128, R], F32, tag="pa")
            for k in range(8):
                nc.tensor.matmul(
                    out=pb[:],
                    lhsT=x_sb[:, k, 128 * m:128 * (m + 1)],
                    rhs=wh_sb[:, k, :],
                    start=(k == 0), stop=(k == 7),
                )
            sb = stage.tile([128, PAD], F32, tag="sa")
            nc.vector.memset(sb[:, R:PAD], 0.0)
            nc.vector.tensor_copy(out=sb[:, 0:R], in_=pb[:])
            w = nc.gpsimd.dma_start(
                out=b_dram[128 * m:128 * (m + 1), :], in_=sb[:]
            )
            table_writes.append(w)

        # --- gather + reduce --------------------------------------------
        from concourse.tile_rust import add_dep_helper

        for g in range(ROUNDS // GRP):
            ga = gap.tile([128, GRP * CHUNK, PAD], F32, tag="ga")
            gb = gbp.tile([128, GRP * CHUNK, PAD], F32, tag="gb")
            for rr in range(GRP):
                r = g * GRP + rr
                isl = slice(r * (NI // 16), (r + 1) * (NI // 16))
                dsl = slice(rr * CHUNK, (rr + 1) * CHUNK)
                g1 = nc.gpsimd.dma_gather(
                    ga[:, dsl, :], a_dram[:], isb_s[:, isl], NI, NI, PAD,
                    queue_num=(2 * r) % 4,
                )
                g2 = nc.gpsimd.dma_gather(
                    gb[:, dsl, :], b_dram[:], isb_h[:, isl], NI, NI, PAD,
                    queue_num=(2 * r + 1) % 4,
                )
            prod = prodp.tile([128, GRP * CHUNK, R], F32, tag="prod")
            nc.vector.tensor_mul(prod[:], ga[:, :, 0:R], gb[:, :, 0:R])
            nc.vector.tensor_reduce(
                out=out_sb[:, g * GRP * CHUNK:(g + 1) * GRP * CHUNK],
                in_=prod[:],
                axis=mybir.AxisListType.X,
                op=mybir.AluOpType.add,
            )
        nc.sync.dma_start(out=out[:], in_=out_sb[:])


_nc_cache_by_reps = {}


def _get_nc(reps: int = 1):
    nc = _nc_cache_by_reps.get(reps)
    if nc is None:
        nc = _nc_cache_by_reps[reps] = _build(reps)
    return nc


def prepare_in_maps(hidden_states, W_seq, W_hid, all_indices):
    x_bf = [np.ascontiguousarray(hidden_states[b].astype(ml_dtypes.bfloat16))
            for b in range(B)]
    ws_t = np.ascontiguousarray(W_seq.T.astype(ml_dtypes.bfloat16))
    wh_t = np.ascontiguousarray(W_hid.T.astype(ml_dtypes.bfloat16))
    idx_pairs = []
    for q in range(4):
        seg = all_indices[q * J:(q + 1) * J]
        idx_pairs.append((_wrap_idx(seg[:, 0]), _wrap_idx(seg[:, 1])))
    in_maps = []
    for c in range(NCORES):
        b, q = c // 4, c % 4
        in_maps.append({
            "x": x_bf[b],
            "wseq_t": ws_t,
            "whid_t": wh_t,
            "idx_s": idx_pairs[q][0],
            "idx_h": idx_pairs[q][1],
        })
    return in_maps


def _wrap_idx(v: np.ndarray) -> np.ndarray:
    """[J] int -> [16, J/16] int16 in dma_gather wrapped layout.

    Round r (NI indices) occupies columns [r*NI/16, (r+1)*NI/16); within a
    round index j sits at [j % 16, j // 16].
    """
    w = v.astype(np.int16).reshape(ROUNDS, NI // 16, 16)
    w = w.transpose(2, 0, 1).reshape(16, J // 16)
    return np.ascontiguousarray(w)


def kernel(hidden_states, W_seq, W_hid, all_indices):
    hidden_states = np.asarray(hidden_states)
    W_seq = np.asarray(W_seq)
    W_hid = np.asarray(W_hid)
    all_indices = np.asarray(all_indices)

    nc = _get_nc()
    in_maps = prepare_in_maps(hidden_states, W_seq, W_hid, all_indices)
    res = run_bass_kernel_spmd(nc, in_maps, list(range(NCORES)))

    out = np.empty((B, N), dtype=np.float32)
    for c in range(NCORES):
        b, q = c // 4, c % 4
        o = res.results[c]["out"].reshape(128, ROUNDS, CHUNK)
        # out_sb[p, r, ch] holds output j = r*NI + ch*128 + p
        out[b, q * J:(q + 1) * J] = o.transpose(1, 2, 0).reshape(J)
    return out.reshape(B, S, H)


# revision 13
# speedup vs baseline: 70.5671x; 6.1079x over previous
"""Trainium2 Bass kernel for nn_CPCircuitLayer (embedding_lookup).

Math: A = X @ W_seq^T  [S,R];  Bm = X^T @ W_hid^T  [H,R]
      out[b, n] = dot(A[b, idx_s[n]], Bm[b, idx_h[n]]),  out -> [B, S, H]

Sharding (8 cores, no collectives): core c handles batch b = c//4 and the
quarter q = c%4 of the N = S*H index list (J = N/4 indices, both factor
tables computed redundantly per batch group from the full X[b]).

Per-core device pipeline:
  1. Load X[b] (bf16) + transposed copy via HWDGE transpose-DMA.
  2. PE matmuls (bf16 in, f32 psum): A rows and Bm rows -> DRAM tables
     [1024, 64] f32 (only first 32 cols valid; 256B row stride).
  3. dma_gather (SWDGE ucode): for each round of 1024 indices, gather the
     256B rows of A and Bm picked by idx_s / idx_h.
  4. DVE: prod = ga * gb over the valid 32 lanes, segmented reduce -> out.
"""

import os
import numpy as np
import ml_dtypes
from contextlib import ExitStack

import concourse.bass as bass
import concourse.mybir as mybir
import concourse.tile as tile
from concourse import bacc
from concourse.bass_utils import run_bass_kernel_spmd

B, S, H, R = 2, 1024, 1024, 32
N = S * H
NCORES = 8
J = N // 4            # outputs per core (one batch, quarter of N) = 262144
NI = 1024             # indices per dma_gather (ring-capacity limit ~1024)
ROUNDS = J // NI      # 256
GRP = 8               # rounds per DVE mul/reduce group
CHUNK = NI // 128     # dst chunks per round = 8
PAD = 64              # table row f32 elems (256B rows)

F32 = mybir.dt.float32
BF16 = mybir.dt.bfloat16
I16 = mybir.dt.int16

_nc_cache = None


def _build(reps: int = 1):
    nc = bacc.Bacc(num_swdge_queues=4)
    x = nc.declare_dram_parameter("x", [S, H], BF16, False)
    wseq_t = nc.declare_dram_parameter("wseq_t", [H, R], BF16, False)
    whid_t = nc.declare_dram_parameter("whid_t", [S, R], BF16, False)
    idx_s = nc.declare_dram_parameter("idx_s", [16, J // 16], I16, False)
    idx_h = nc.declare_dram_parameter("idx_h", [16, J // 16], I16, False)
    out = nc.declare_dram_parameter("out", [128, ROUNDS * CHUNK], F32, True)
    a_dram = nc.dram_tensor("a_rows", [S, PAD], F32)
    b_dram = nc.dram_tensor("b_rows", [H, PAD], F32)

    with tile.TileContext(nc) as tc, ExitStack() as ctx:
        base = ctx.enter_context(tc.tile_pool(name="base", bufs=1))
        psum = ctx.enter_context(tc.tile_pool(name="psum", bufs=2, space="PSUM"))
        stage = ctx.enter_context(tc.tile_pool(name="stage", bufs=2))
        gap = ctx.enter_context(tc.tile_pool(name="gap", bufs=2))
        gbp = ctx.enter_context(tc.tile_pool(name="gbp", bufs=2))
        prodp = ctx.enter_context(tc.tile_pool(name="prodp", bufs=2))

        # --- loads -------------------------------------------------------
        x_sb = base.tile([128, 8, H], BF16)       # X[s,h]: p=s%128, k=s//128
        xt_sb = base.tile([128, 8, S], BF16)      # X^T[h,s]: p=h%128, k=h//128
        ws_sb = base.tile([128, 8, R], BF16)      # W_seq^T rows (h-major)
        wh_sb = base.tile([128, 8, R], BF16)      # W_hid^T rows (s-major)
        isb_s = base.tile([128, J // 16], I16)
        isb_h = base.tile([128, J // 16], I16)
        out_sb = base.tile([128, ROUNDS * CHUNK], F32)

        nc.sync.dma_start(
            out=x_sb[:],
            in_=bass.AP(tensor=x[:].tensor, offset=0,
                        ap=[[H, 128], [128 * H, 8], [1, H]]),
        )
        for k in range(8):
            nc.sync.dma_start_transpose(
                out=xt_sb[:, k, :], in_=x[:, 128 * k:128 * (k + 1)]
            )
        nc.sync.dma_start(
            out=ws_sb[:],
            in_=bass.AP(tensor=wseq_t[:].tensor, offset=0,
                        ap=[[R, 128], [128 * R, 8], [1, R]]),
        )
        nc.sync.dma_start(
            out=wh_sb[:],
            in_=bass.AP(tensor=whid_t[:].tensor, offset=0,
                        ap=[[R, 128], [128 * R, 8], [1, R]]),
        )
        # idx: replicate [16, J/16] across the 8 partition groups
        nc.sync.dma_start(
            out=isb_s[:],
            in_=bass.AP(tensor=idx_s[:].tensor, offset=0,
                        ap=[[0, 8], [J // 16, 16], [1, J // 16]]),
        )
        nc.sync.dma_start(
            out=isb_h[:],
            in_=bass.AP(tensor=idx_h[:].tensor, offset=0,
                        ap=[[0, 8], [J // 16, 16], [1, J // 16]]),
        )

        # --- body (repeatable for HW-time measurement) -------------------
        for _rep in range(reps):
            _body(nc, tc, base, psum, stage, gap, gbp, prodp,
                  x_sb, xt_sb, ws_sb, wh_sb, isb_s, isb_h, out_sb,
                  a_dram, b_dram, out)
    nc.compile()
    return nc


def _body(nc, tc, base, psum, stage, gap, gbp, prodp,
          x_sb, xt_sb, ws_sb, wh_sb, isb_s, isb_h, out_sb,
          a_dram, b_dram, out):
    if True:
        # A rows: A[s, r] = sum_h X[s, h] Wseq^T[h, r]
        #   lhsT = X^T tile [128h, 128s], rhs = Wseq^T tile [128h, 32]
        table_writes = []
        for m in range(8):
            pa = psum.tile([128, R], F32, tag="pa")
            for k in range(8):
                nc.tensor.matmul(
                    out=pa[:],
                    lhsT=xt_sb[:, k, 128 * m:128 * (m + 1)],
                    rhs=ws_sb[:, k, :],
                    start=(k == 0), stop=(k == 7),
                )
            sa = stage.tile([128, PAD], F32, tag="sa")
            nc.vector.memset(sa[:, R:PAD], 0.0)
            nc.vector.tensor_copy(out=sa[:, 0:R], in_=pa[:])
            w = nc.gpsimd.dma_start(
                out=a_dram[128 * m:128 * (m + 1), :], in_=sa[:]
            )
            table_writes.append(w)
        # B rows: Bm[h, r] = sum_s X[s, h] Whid^T[s, r]
        #   lhsT = X tile [128s, 128h], rhs = Whid^T tile [128s, 32]
        for m in range(8):
            pb = psum.tile([128, R], F32, tag="pa")
            for k in range(8):
                nc.tensor.matmul(
                    out=pb[:],
                    lhsT=x_sb[:, k, 128 * m:128 * (m + 1)],
                    rhs=wh_sb[:, k, :],
                    start=(k == 0), stop=(k == 7),
                )
            sb = stage.tile([128, PAD], F32, tag="sa")
            nc.vector.memset(sb[:, R:PAD], 0.0)
            nc.vector.tensor_copy(out=sb[:, 0:R], in_=pb[:])
            w = nc.gpsimd.dma_start(
                out=b_dram[128 * m:128 * (m + 1), :], in_=sb[:]
            )
            table_writes.append(w)

        # --- gather + reduce --------------------------------------------
        from concourse.tile_rust import add_dep_helper

        for g in range(ROUNDS // GRP):
            ga = gap.tile([128, GRP * CHUNK, PAD], F32, tag="ga")
            gb = gbp.tile([128, GRP * CHUNK, PAD], F32, tag="gb")
            for rr in range(GRP):
                r = g * GRP + rr
                isl = slice(r * (NI // 16), (r + 1) * (NI // 16))
                dsl = slice(rr * CHUNK, (rr + 1) * CHUNK)
                g1 = nc.gpsimd.dma_gather(
                    ga[:, dsl, :], a_dram[:], isb_s[:, isl], NI, NI, PAD,
                    queue_num=(2 * r) % 4,
                )
                g2 = nc.gpsimd.dma_gather(
                    gb[:, dsl, :], b_dram[:], isb_h[:, isl], NI, NI, PAD,
                    queue_num=(2 * r + 1) % 4,
                )
            prod = prodp.tile([128, GRP * CHUNK, R], F32, tag="prod")
            nc.vector.tensor_mul(prod[:], ga[:, :, 0:R], gb[:, :, 0:R])
            nc.vector.tensor_reduce(
                out=out_sb[:, g * GRP * CHUNK:(g + 1) * GRP * CHUNK],
                in_=prod[:],
                axis=mybir.AxisListType.X,
                op=mybir.AluOpType.add,
            )
        nc.sync.dma_start(out=out[:], in_=out_sb[:])


_nc_cache_by_reps = {}


def _get_nc(reps: int = 1):
    nc = _nc_cache_by_reps.get(reps)
    if nc is None:
        nc = _nc_cache_by_reps[reps] = _build(reps)
    return nc


class _Runner:
    """Trace/compile the SPMD executable once; reuse across calls.

    run_bass_kernel_spmd re-traces (and re-runs the walrus pipeline) on
    every invocation; this caches the jitted shard_map callable keyed on
    the Bass graph.
    """

    def __init__(self, nc):
        import jax
        from jax.experimental.shard_map import shard_map
        from jax.sharding import Mesh, PartitionSpec
        import concourse.bass2jax as b2j

        b2j.install_neuronx_cc_hook()
        self.nc = nc
        part_name = (nc.partition_id_tensor.name
                     if nc.partition_id_tensor else None)
        in_names, out_names, out_avals = [], [], []
        zero_outs = []
        for alloc in nc.m.functions[0].allocations:
            if not isinstance(alloc, mybir.MemoryLocationSet):
                continue
            name = alloc.memorylocations[0].name
            if alloc.kind == "ExternalInput":
                if name != part_name:
                    in_names.append(name)
            elif alloc.kind == "ExternalOutput":
                out_names.append(name)
                shape = tuple(alloc.tensor_shape)
                dtype = mybir.dt.np(alloc.dtype)
                out_avals.append(jax.core.ShapedArray(shape, dtype))
                zero_outs.append(np.zeros(shape, dtype))
        self.in_names = list(in_names)
        self.out_names = out_names
        self.zero_outs = zero_outs
        n_params = len(in_names)
        n_outs = len(out_names)
        all_in_names = in_names + out_names
        if part_name is not None:
            all_in_names = all_in_names + [part_name]
        donate = tuple(range(n_params, n_params + n_outs))

        def _body(*args):
            operands = list(args)
            if part_name is not None:
                operands.append(b2j.partition_id_tensor())
            outs = b2j._bass_exec_p.bind(
                *operands,
                out_avals=tuple(out_avals),
                in_names=tuple(all_in_names),
                out_names=tuple(out_names),
                lowering_input_output_aliases=(),
                sim_require_finite=True,
                sim_require_nnan=True,
                nc=nc,
            )
            return tuple(outs)

        devices = jax.devices()[:NCORES]
        mesh = Mesh(np.asarray(devices), ("core",))
        self.fn = jax.jit(
            shard_map(
                _body, mesh=mesh,
                in_specs=(PartitionSpec("core"),) * (n_params + n_outs),
                out_specs=(PartitionSpec("core"),) * n_outs,
                check_rep=False,
            ),
            donate_argnums=donate,
            keep_unused=True,
        )
        self.n_params = n_params

    def __call__(self, in_maps):
        concat_in = [
            np.concatenate([np.asarray(m[name]) for m in in_maps], axis=0)
            for name in self.in_names
        ]
        concat_zeros = [
            np.zeros((NCORES * z.shape[0], *z.shape[1:]), z.dtype)
            for z in self.zero_outs
        ]
        out_arrs = self.fn(*concat_in, *concat_zeros)
        return [
            {
                name: np.asarray(out_arrs[i]).reshape(NCORES, -1)[c]
                for i, name in enumerate(self.out_names)
            }
            for c in range(NCORES)
        ]


_runner_cache = {}


def _get_runner(reps: int = 1):
    r = _runner_cache.get(reps)
    if r is None:
        r = _runner_cache[reps] = _Runner(_get_nc(reps))
    return r


def prepare_in_maps(hidden_states, W_seq, W_hid, all_indices):
    x_bf = [np.ascontiguousarray(hidden_states[b].astype(ml_dtypes.bfloat16))
            for b in range(B)]
    ws_t = np.ascontiguousarray(W_seq.T.astype(ml_dtypes.bfloat16))
    wh_t = np.ascontiguousarray(W_hid.T.astype(ml_dtypes.bfloat16))
    idx_pairs = []
    for q in range(4):
        seg = all_indices[q * J:(q + 1) * J]
        idx_pairs.append((_wrap_idx(seg[:, 0]), _wrap_idx(seg[:, 1])))
    in_maps = []
    for c in range(NCORES):
        b, q = c // 4, c % 4
        in_maps.append({
            "x": x_bf[b],
            "wseq_t": ws_t,
            "whid_t": wh_t,
            "idx_s": idx_pairs[q][0],
            "idx_h": idx_pairs[q][1],
        })
    return in_maps


def _wrap_idx(v: np.ndarray) -> np.ndarray:
    """[J] int -> [16, J/16] int16 in dma_gather wrapped layout.

    Round r (NI indices) occupies columns [r*NI/16, (r+1)*NI/16); within a
    round index j sits at [j % 16, j // 16].
    """
    w = v.astype(np.int16).reshape(ROUNDS, NI // 16, 16)
    w = w.transpose(2, 0, 1).reshape(16, J // 16)
    return np.ascontiguousarray(w)


def kernel(hidden_states, W_seq, W_hid, all_indices):
    hidden_states = np.asarray(hidden_states)
    W_seq = np.asarray(W_seq)
    W_hid = np.asarray(W_hid)
    all_indices = np.asarray(all_indices)

    runner = _get_runner()
    in_maps = prepare_in_maps(hidden_states, W_seq, W_hid, all_indices)
    results = runner(in_maps)

    out = np.empty((B, N), dtype=np.float32)
    for c in range(NCORES):
        b, q = c // 4, c % 4
        o = results[c]["out"].reshape(128, ROUNDS, CHUNK)
        # out_sb[p, r, ch] holds output j = r*NI + ch*128 + p
        out[b, q * J:(q + 1) * J] = o.transpose(1, 2, 0).reshape(J)
    return out.reshape(B, S, H)


# revision 21
# speedup vs baseline: 74.3367x; 1.0534x over previous
"""Trainium2 Bass kernel for nn_CPCircuitLayer (embedding_lookup).

Math: A = X @ W_seq^T  [S,R];  Bm = X^T @ W_hid^T  [H,R]
      out[b, n] = dot(A[b, idx_s[n]], Bm[b, idx_h[n]]),  out -> [B, S, H]

Sharding (8 cores, no collectives): core c handles batch b = c//4 and the
quarter q = c%4 of the N = S*H index list (J = N/4 indices). Both factor
tables are computed redundantly per batch group from the full X[b].

Per-core device pipeline:
  1. Load X[b] (bf16) + transposed copy via HWDGE transpose-DMA.
  2. PE matmuls (bf16 in, f32 psum): A^T and Bm^T [32, 1024].
  3. Repack to per-lane split-R tables: partition p holds columns
     2*(p%16), 2*(p%16)+1 of the factor interleaved ([128, 1024, 2] f32),
     via a DRAM bounce + 8x partition-group broadcast load.
  4. ap_gather (GPSIMD FIFO): each 16-partition group streams its own
     indices; one instruction gathers NIdx rows x 8 groups.
  5. DVE mul + pair-sum, then PE block-indicator matmul reduces the 16
     lanes x 2 of each group -> psum [8, n] -> out.
"""

import numpy as np
import ml_dtypes
from contextlib import ExitStack

import concourse.bass as bass
import concourse.mybir as mybir
import concourse.tile as tile
from concourse import bacc

B, S, H, R = 2, 1024, 1024, 32
N = S * H
NCORES = 8
J = N // 4            # outputs per core (one batch, quarter of N) = 262144
JG = J // 8           # outputs per 16-partition group = 32768
NIdx = 2048           # indices per group per ap_gather instruction
RNDS = JG // NIdx     # 16 gather rounds per table
GRP_D = 2             # table f32 per lane (R = 16 lanes * 2)

F32 = mybir.dt.float32
BF16 = mybir.dt.bfloat16
I16 = mybir.dt.int16


def _build(reps: int = 1):
    nc = bacc.Bacc()
    x = nc.declare_dram_parameter("x", [S, H], BF16, False)
    wseq_t = nc.declare_dram_parameter("wseq_t", [H, R], BF16, False)
    whid_t = nc.declare_dram_parameter("whid_t", [S, R], BF16, False)
    # per-group index streams, wrapped: group g's jj-th index lives at
    # [16*g + jj%16, jj//16]
    idx_s = nc.declare_dram_parameter("idx_s", [128, 2 * JG // 16], I16, False)
    idx_h = nc.declare_dram_parameter("idx_h", [128, 2 * JG // 16], I16, False)
    ind_in = nc.declare_dram_parameter("ind", [128, 8], F32, False)
    out = nc.declare_dram_parameter("out", [8, JG], F32, True)
    ta_dram = nc.dram_tensor("ta", [R, S], F32)   # A^T bounce
    tb_dram = nc.dram_tensor("tb", [R, H], F32)   # Bm^T bounce

    with tile.TileContext(nc) as tc, ExitStack() as ctx:
        base = ctx.enter_context(tc.tile_pool(name="base", bufs=1))
        psum = ctx.enter_context(tc.tile_pool(name="psum", bufs=2, space="PSUM"))
        rpsum = ctx.enter_context(tc.tile_pool(name="rpsum", bufs=2, space="PSUM"))
        stage = ctx.enter_context(tc.tile_pool(name="stage", bufs=2))
        gap = ctx.enter_context(tc.tile_pool(name="gap", bufs=2))
        gbp = ctx.enter_context(tc.tile_pool(name="gbp", bufs=2))
        prodp = ctx.enter_context(tc.tile_pool(name="prodp", bufs=2))

        # --- loads -------------------------------------------------------
        x_sb = base.tile([128, 8, H], BF16)       # X[s,h]: p=s%128, k=s//128
        xt_sb = base.tile([128, 8, S], BF16)      # X^T[h,s]: p=h%128, k=h//128
        ws_sb = base.tile([128, 8, R], BF16)      # W_seq^T rows (h-major)
        wh_sb = base.tile([128, 8, R], BF16)      # W_hid^T rows (s-major)
        isb_s = base.tile([128, 2 * JG // 16], I16)
        isb_h = base.tile([128, 2 * JG // 16], I16)
        ind_sb = base.tile([128, 8], F32)         # block indicator for reduce
        ta_sb = base.tile([128, 2 * S], F32)
        tb_sb = base.tile([128, 2 * H], F32)

        nc.sync.dma_start(
            out=x_sb[:],
            in_=bass.AP(tensor=x[:].tensor, offset=0,
                        ap=[[H, 128], [128 * H, 8], [1, H]]),
        )
        for k in range(8):
            nc.sync.dma_start_transpose(
                out=xt_sb[:, k, :], in_=x[:, 128 * k:128 * (k + 1)]
            )
        nc.sync.dma_start(
            out=ws_sb[:],
            in_=bass.AP(tensor=wseq_t[:].tensor, offset=0,
                        ap=[[R, 128], [128 * R, 8], [1, R]]),
        )
        nc.sync.dma_start(
            out=wh_sb[:],
            in_=bass.AP(tensor=whid_t[:].tensor, offset=0,
                        ap=[[R, 128], [128 * R, 8], [1, R]]),
        )
        nc.sync.dma_start(out=isb_s[:], in_=idx_s[:])
        nc.sync.dma_start(out=isb_h[:], in_=idx_h[:])

        nc.sync.dma_start(out=ind_sb[:], in_=ind_in[:])

        for _ in range(reps):
            _body(nc, psum, rpsum, stage, gap, gbp, prodp,
                  x_sb, xt_sb, ws_sb, wh_sb, isb_s, isb_h, ind_sb,
                  ta_sb, tb_sb, ta_dram, tb_dram, out)
    nc.compile()
    return nc


def _body(nc, psum, rpsum, stage, gap, gbp, prodp,
          x_sb, xt_sb, ws_sb, wh_sb, isb_s, isb_h, ind_sb,
          ta_sb, tb_sb, ta_dram, tb_dram, out):
    # --- factor transposes on PE: F^T [32, 1024] ------------------------
    # A^T[r, s] = sum_h Wseq^T[h, r] X^T[h, s]; Bm^T[r, h] = sum_s ...
    for (tdram, lhs_w, rhs_x) in ((ta_dram, ws_sb, xt_sb),
                                  (tb_dram, wh_sb, x_sb)):
        ft = stage.tile([R, S], F32, tag="ft")
        for nh in range(2):
            pt = psum.tile([R, S // 2], F32, tag="pt")
            for k in range(8):
                nc.tensor.matmul(
                    out=pt[:],
                    lhsT=lhs_w[:, k, :],
                    rhs=rhs_x[:, k, nh * 512:(nh + 1) * 512],
                    start=(k == 0), stop=(k == 7),
                )
            nc.vector.tensor_copy(out=ft[:, nh * 512:(nh + 1) * 512], in_=pt[:])
        nc.gpsimd.dma_start(out=tdram[:], in_=ft[:])

    # broadcast tables back, lane-split d=2 interleaved: partition p
    # (lane l = p%16) holds tab[p, v, d] = F^T[2l+d, v]
    for (tdram, tsb, V) in ((ta_dram, ta_sb, S), (tb_dram, tb_sb, H)):
        nc.gpsimd.dma_start(
            out=tsb[:],
            in_=bass.AP(tensor=tdram[:].tensor, offset=0,
                        ap=[[0, 8], [2 * V, 16], [1, 2 * V]]),
        )

    # --- gather + reduce ------------------------------------------------
    for rnd in range(RNDS):
        isl = slice(rnd * (2 * NIdx // 16), (rnd + 1) * (2 * NIdx // 16))
        ga = gap.tile([128, NIdx, GRP_D], F32, tag="ga")
        gb = gbp.tile([128, NIdx, GRP_D], F32, tag="gb")
        ga_flat = bass.AP(tensor=ga[:].tensor, offset=ga[:].offset,
                          ap=[list(ga[:].ap[0]), [1, 2 * NIdx], [1, 1]])
        gb_flat = bass.AP(tensor=gb[:].tensor, offset=gb[:].offset,
                          ap=[list(gb[:].ap[0]), [1, 2 * NIdx], [1, 1]])
        nc.gpsimd.ap_gather(
            out_ap=ga_flat, in_ap=ta_sb[:], idxs_ap=isb_s[:, isl],
            channels=128, num_elems=2 * S, d=1, num_idxs=2 * NIdx,
        )
        nc.gpsimd.ap_gather(
            out_ap=gb_flat, in_ap=tb_sb[:], idxs_ap=isb_h[:, isl],
            channels=128, num_elems=2 * H, d=1, num_idxs=2 * NIdx,
        )
        prod = prodp.tile([128, NIdx, GRP_D], F32, tag="prod")
        nc.vector.tensor_mul(prod[:], ga[:], gb[:])
        p2 = prodp.tile([128, NIdx], F32, tag="p2")
        nc.vector.tensor_add(p2[:], prod[:, :, 0], prod[:, :, 1])
        # reduce 16 lanes per group via block-indicator matmul
        ot = prodp.tile([8, NIdx], F32, tag="ot")
        for t in range(NIdx // 512):
            rp = rpsum.tile([8, 512], F32, tag="rp")
            nc.tensor.matmul(
                out=rp[:],
                lhsT=ind_sb[:],
                rhs=p2[:, t * 512:(t + 1) * 512],
                start=True, stop=True,
            )
            nc.scalar.copy(out=ot[:, t * 512:(t + 1) * 512], in_=rp[:])
        nc.sync.dma_start(
            out=bass.AP(tensor=out[:].tensor, offset=rnd * NIdx,
                        ap=[[JG, 8], [1, NIdx]]),
            in_=ot[:],
        )


_nc_cache_by_reps = {}


def _get_nc(reps: int = 1):
    nc = _nc_cache_by_reps.get(reps)
    if nc is None:
        nc = _nc_cache_by_reps[reps] = _build(reps)
    return nc


class _Runner:
    """Trace/compile the SPMD executable once; reuse across calls."""

    def __init__(self, nc):
        import jax
        from jax.experimental.shard_map import shard_map
        from jax.sharding import Mesh, PartitionSpec
        import concourse.bass2jax as b2j

        b2j.install_neuronx_cc_hook()
        self.nc = nc
        part_name = (nc.partition_id_tensor.name
                     if nc.partition_id_tensor else None)
        in_names, out_names, out_avals = [], [], []
        zero_outs = []
        for alloc in nc.m.functions[0].allocations:
            if not isinstance(alloc, mybir.MemoryLocationSet):
                continue
            name = alloc.memorylocations[0].name
            if alloc.kind == "ExternalInput":
                if name != part_name:
                    in_names.append(name)
            elif alloc.kind == "ExternalOutput":
                out_names.append(name)
                shape = tuple(alloc.tensor_shape)
                dtype = mybir.dt.np(alloc.dtype)
                out_avals.append(jax.core.ShapedArray(shape, dtype))
                zero_outs.append(np.zeros(shape, dtype))
        self.in_names = list(in_names)
        self.out_names = out_names
        self.zero_outs = zero_outs
        n_params = len(in_names)
        n_outs = len(out_names)
        all_in_names = in_names + out_names
        if part_name is not None:
            all_in_names = all_in_names + [part_name]
        donate = tuple(range(n_params, n_params + n_outs))

        def _body_fn(*args):
            operands = list(args)
            if part_name is not None:
                operands.append(b2j.partition_id_tensor())
            outs = b2j._bass_exec_p.bind(
                *operands,
                out_avals=tuple(out_avals),
                in_names=tuple(all_in_names),
                out_names=tuple(out_names),
                lowering_input_output_aliases=(),
                sim_require_finite=True,
                sim_require_nnan=True,
                nc=nc,
            )
            return tuple(outs)

        devices = jax.devices()[:NCORES]
        mesh = Mesh(np.asarray(devices), ("core",))
        self.fn = jax.jit(
            shard_map(
                _body_fn, mesh=mesh,
                in_specs=(PartitionSpec("core"),) * (n_params + n_outs),
                out_specs=(PartitionSpec("core"),) * n_outs,
                check_rep=False,
            ),
            donate_argnums=donate,
            keep_unused=True,
        )

    def __call__(self, in_maps):
        concat_in = [
            np.concatenate([np.asarray(m[name]) for m in in_maps], axis=0)
            for name in self.in_names
        ]
        concat_zeros = [
            np.zeros((NCORES * z.shape[0], *z.shape[1:]), z.dtype)
            for z in self.zero_outs
        ]
        out_arrs = self.fn(*concat_in, *concat_zeros)
        return [
            {
                name: np.asarray(out_arrs[i]).reshape(NCORES, -1)[c]
                for i, name in enumerate(self.out_names)
            }
            for c in range(NCORES)
        ]


_runner_cache = {}


def _get_runner(reps: int = 1):
    r = _runner_cache.get(reps)
    if r is None:
        r = _runner_cache[reps] = _Runner(_get_nc(reps))
    return r


def _wrap_idx(v: np.ndarray) -> np.ndarray:
    """[J] -> [128, 2*JG/16] int16: group g = j // JG streams the pairs
    (v, v+1024) for its outputs, wrapped at [16*g + t%16, t//16]."""
    v = v.astype(np.int16)
    v2 = np.empty(2 * J, np.int16)
    v2[0::2] = v
    v2[1::2] = v + 1024
    w = v2.reshape(8, 2 * JG // 16, 16)   # [g, col, p16]
    w = w.transpose(0, 2, 1).reshape(128, 2 * JG // 16)
    return np.ascontiguousarray(w)


def prepare_in_maps(hidden_states, W_seq, W_hid, all_indices):
    x_bf = [np.ascontiguousarray(hidden_states[b].astype(ml_dtypes.bfloat16))
            for b in range(B)]
    ws_t = np.ascontiguousarray(W_seq.T.astype(ml_dtypes.bfloat16))
    wh_t = np.ascontiguousarray(W_hid.T.astype(ml_dtypes.bfloat16))
    idx_pairs = []
    for q in range(4):
        seg = all_indices[q * J:(q + 1) * J]
        idx_pairs.append((_wrap_idx(seg[:, 0]), _wrap_idx(seg[:, 1])))
    in_maps = []
    for c in range(NCORES):
        b, q = c // 4, c % 4
        ind = np.zeros((128, 8), np.float32)
        for g in range(8):
            ind[16 * g:16 * (g + 1), g] = 1.0
        in_maps.append({
            "x": x_bf[b],
            "wseq_t": ws_t,
            "whid_t": wh_t,
            "idx_s": idx_pairs[q][0],
            "idx_h": idx_pairs[q][1],
            "ind": ind,
        })
    return in_maps


def kernel(hidden_states, W_seq, W_hid, all_indices):
    hidden_states = np.asarray(hidden_states)
    W_seq = np.asarray(W_seq)
    W_hid = np.asarray(W_hid)
    all_indices = np.asarray(all_indices)

    runner = _get_runner()
    in_maps = prepare_in_maps(hidden_states, W_seq, W_hid, all_indices)
    results = runner(in_maps)

    out = np.empty((B, N), dtype=np.float32)
    for c in range(NCORES):
        b, q = c // 4, c % 4
        o = results[c]["out"].reshape(8, JG)
        # out[g, jj] holds output j = g*JG + jj of this core's quarter
        out[b, q * J:(q + 1) * J] = o.reshape(J)
    return out.reshape(B, S, H)


# revision 24
# speedup vs baseline: 115.5451x; 1.5543x over previous
"""Trainium2 Bass kernel for nn_CPCircuitLayer (embedding_lookup).

Math: A = X @ W_seq^T  [S,R];  Bm = X^T @ W_hid^T  [H,R]
      out[b, n] = dot(A[b, idx_s[n]], Bm[b, idx_h[n]]),  out -> [B, S, H]

Sharding (8 cores, no collectives): core c handles batch b = c//4 and the
quarter q = c%4 of the N = S*H index list (J = N/4 indices). Both factor
tables are computed redundantly per batch group from the full X[b].

Per-core device pipeline:
  1. Load X[b] (bf16) + transposed copy via HWDGE transpose-DMA.
  2. PE matmuls (bf16 in, f32 psum): A^T and Bm^T [32, 1024].
  3. Repack to per-lane split-R tables: partition p holds columns
     2*(p%16), 2*(p%16)+1 of the factor interleaved ([128, 1024, 2] f32),
     via a DRAM bounce + 8x partition-group broadcast load.
  4. ap_gather (GPSIMD FIFO): each 16-partition group streams its own
     indices; one instruction gathers NIdx rows x 8 groups.
  5. DVE mul + pair-sum, then PE block-indicator matmul reduces the 16
     lanes x 2 of each group -> psum [8, n] -> out.
"""

import numpy as np
import ml_dtypes
from contextlib import ExitStack

import concourse.bass as bass
import concourse.mybir as mybir
import concourse.tile as tile
from concourse import bacc

B, S, H, R = 2, 1024, 1024, 32
N = S * H
NCORES = 8
J = N // 4            # outputs per core (one batch, quarter of N) = 262144
JG = J // 8           # outputs per 16-partition group = 32768
NIdx = 2048           # indices per group per ap_gather instruction
RNDS = JG // NIdx     # 16 gather rounds per table
GRP_D = 2             # table f32 per lane (R = 16 lanes * 2)
SKIP_GATHER = False   # timing experiment: drop ap_gather instructions

F32 = mybir.dt.float32
BF16 = mybir.dt.bfloat16
I16 = mybir.dt.int16


def _build(reps: int = 1):
    nc = bacc.Bacc()
    x = nc.declare_dram_parameter("x", [S, H], BF16, False)
    wseq_t = nc.declare_dram_parameter("wseq_t", [H, R], BF16, False)
    whid_t = nc.declare_dram_parameter("whid_t", [S, R], BF16, False)
    # per-group index streams, wrapped: group g's jj-th index lives at
    # [16*g + jj%16, jj//16]
    idx_s = nc.declare_dram_parameter("idx_s", [128, 2 * JG // 16], I16, False)
    idx_h = nc.declare_dram_parameter("idx_h", [128, 2 * JG // 16], I16, False)
    ind_in = nc.declare_dram_parameter("ind", [128, 8], F32, False)
    out = nc.declare_dram_parameter("out", [8, JG], F32, True)
    ta_dram = nc.dram_tensor("ta", [R, S], F32)   # A^T bounce
    tb_dram = nc.dram_tensor("tb", [R, H], F32)   # Bm^T bounce

    with tile.TileContext(nc) as tc, ExitStack() as ctx:
        base = ctx.enter_context(tc.tile_pool(name="base", bufs=1))
        psum = ctx.enter_context(tc.tile_pool(name="psum", bufs=2, space="PSUM"))
        rpsum = ctx.enter_context(tc.tile_pool(name="rpsum", bufs=1, space="PSUM"))
        stage = ctx.enter_context(tc.tile_pool(name="stage", bufs=2))
        gap = ctx.enter_context(tc.tile_pool(name="gap", bufs=2))
        gbp = ctx.enter_context(tc.tile_pool(name="gbp", bufs=2))
        prodp = ctx.enter_context(tc.tile_pool(name="prodp", bufs=2))

        # --- loads -------------------------------------------------------
        x_sb = base.tile([128, 8, H], BF16)       # X[s,h]: p=s%128, k=s//128
        xt_sb = base.tile([128, 8, S], BF16)      # X^T[h,s]: p=h%128, k=h//128
        ws_sb = base.tile([128, 8, R], BF16)      # W_seq^T rows (h-major)
        wh_sb = base.tile([128, 8, R], BF16)      # W_hid^T rows (s-major)
        isb_s = base.tile([128, 2 * JG // 16], I16)
        isb_h = base.tile([128, 2 * JG // 16], I16)
        ind_sb = base.tile([128, 8], F32)         # block indicator for reduce
        ta_sb = base.tile([128, 2 * S], F32)
        tb_sb = base.tile([128, 2 * H], F32)

        nc.sync.dma_start(
            out=x_sb[:],
            in_=bass.AP(tensor=x[:].tensor, offset=0,
                        ap=[[H, 128], [128 * H, 8], [1, H]]),
        )
        for k in range(8):
            nc.sync.dma_start_transpose(
                out=xt_sb[:, k, :], in_=x[:, 128 * k:128 * (k + 1)]
            )
        nc.sync.dma_start(
            out=ws_sb[:],
            in_=bass.AP(tensor=wseq_t[:].tensor, offset=0,
                        ap=[[R, 128], [128 * R, 8], [1, R]]),
        )
        nc.sync.dma_start(
            out=wh_sb[:],
            in_=bass.AP(tensor=whid_t[:].tensor, offset=0,
                        ap=[[R, 128], [128 * R, 8], [1, R]]),
        )
        nc.sync.dma_start(out=isb_s[:], in_=idx_s[:])
        nc.sync.dma_start(out=isb_h[:], in_=idx_h[:])

        nc.sync.dma_start(out=ind_sb[:], in_=ind_in[:])

        for _ in range(reps):
            _body(nc, psum, rpsum, stage, gap, gbp, prodp,
                  x_sb, xt_sb, ws_sb, wh_sb, isb_s, isb_h, ind_sb,
                  ta_sb, tb_sb, ta_dram, tb_dram, out)
    nc.compile()
    return nc


def _body(nc, psum, rpsum, stage, gap, gbp, prodp,
          x_sb, xt_sb, ws_sb, wh_sb, isb_s, isb_h, ind_sb,
          ta_sb, tb_sb, ta_dram, tb_dram, out):
    # --- factor transposes on PE: F^T [32, 1024] ------------------------
    # A^T[r, s] = sum_h Wseq^T[h, r] X^T[h, s]; Bm^T[r, h] = sum_s ...
    for (tdram, lhs_w, rhs_x) in ((ta_dram, ws_sb, xt_sb),
                                  (tb_dram, wh_sb, x_sb)):
        ft = stage.tile([R, S], F32, tag="ft")
        for nh in range(2):
            pt = psum.tile([R, S // 2], F32, tag="pt")
            for k in range(8):
                nc.tensor.matmul(
                    out=pt[:],
                    lhsT=lhs_w[:, k, :],
                    rhs=rhs_x[:, k, nh * 512:(nh + 1) * 512],
                    start=(k == 0), stop=(k == 7),
                )
            nc.vector.tensor_copy(out=ft[:, nh * 512:(nh + 1) * 512], in_=pt[:])
        nc.gpsimd.dma_start(out=tdram[:], in_=ft[:])

    # broadcast tables back, lane-split d=2 interleaved: partition p
    # (lane l = p%16) holds tab[p, v, d] = F^T[2l+d, v]
    for (tdram, tsb, V) in ((ta_dram, ta_sb, S), (tb_dram, tb_sb, H)):
        nc.gpsimd.dma_start(
            out=tsb[:],
            in_=bass.AP(tensor=tdram[:].tensor, offset=0,
                        ap=[[0, 8], [2 * V, 16], [1, 2 * V]]),
        )

    # --- gather + reduce ------------------------------------------------
    for rnd in range(RNDS):
        isl = slice(rnd * (2 * NIdx // 16), (rnd + 1) * (2 * NIdx // 16))
        ga = gap.tile([128, NIdx, GRP_D], F32, tag="ga")
        gb = gbp.tile([128, NIdx, GRP_D], F32, tag="gb")
        ga_flat = bass.AP(tensor=ga[:].tensor, offset=ga[:].offset,
                          ap=[list(ga[:].ap[0]), [1, 2 * NIdx], [1, 1]])
        gb_flat = bass.AP(tensor=gb[:].tensor, offset=gb[:].offset,
                          ap=[list(gb[:].ap[0]), [1, 2 * NIdx], [1, 1]])
        if SKIP_GATHER:
            nc.vector.memset(ga[:], 0.0)
            nc.vector.memset(gb[:], 0.0)
        else:
            nc.gpsimd.ap_gather(
                out_ap=ga_flat, in_ap=ta_sb[:], idxs_ap=isb_s[:, isl],
                channels=128, num_elems=2 * S, d=1, num_idxs=2 * NIdx,
            )
            nc.gpsimd.ap_gather(
                out_ap=gb_flat, in_ap=tb_sb[:], idxs_ap=isb_h[:, isl],
                channels=128, num_elems=2 * H, d=1, num_idxs=2 * NIdx,
            )
        prod = prodp.tile([128, NIdx, GRP_D], F32, tag="prod")
        nc.vector.tensor_mul(prod[:], ga[:], gb[:])
        p2 = prodp.tile([128, NIdx], F32, tag="p2")
        nc.vector.tensor_add(p2[:], prod[:, :, 0], prod[:, :, 1])
        # reduce 16 lanes per group via block-indicator matmul; all four
        # 512-col results land in one 4-bank psum tile -> single copy
        ot = prodp.tile([8, NIdx], F32, tag="ot")
        rp4 = rpsum.tile([8, NIdx], F32, tag="rp4")
        for t in range(NIdx // 512):
            nc.tensor.matmul(
                out=rp4[:, t * 512:(t + 1) * 512],
                lhsT=ind_sb[:],
                rhs=p2[:, t * 512:(t + 1) * 512],
                start=True, stop=True,
            )
        nc.scalar.copy(out=ot[:], in_=rp4[:])
        nc.sync.dma_start(
            out=bass.AP(tensor=out[:].tensor, offset=rnd * NIdx,
                        ap=[[JG, 8], [1, NIdx]]),
            in_=ot[:],
        )


_nc_cache_by_reps = {}


def _get_nc(reps: int = 1):
    nc = _nc_cache_by_reps.get(reps)
    if nc is None:
        nc = _nc_cache_by_reps[reps] = _build(reps)
    return nc


class _Runner:
    """Trace/compile the SPMD executable once; reuse across calls."""

    def __init__(self, nc):
        import jax
        from jax.experimental.shard_map import shard_map
        from jax.sharding import Mesh, PartitionSpec
        import concourse.bass2jax as b2j

        b2j.install_neuronx_cc_hook()
        self.nc = nc
        part_name = (nc.partition_id_tensor.name
                     if nc.partition_id_tensor else None)
        in_names, out_names, out_avals = [], [], []
        zero_outs = []
        for alloc in nc.m.functions[0].allocations:
            if not isinstance(alloc, mybir.MemoryLocationSet):
                continue
            name = alloc.memorylocations[0].name
            if alloc.kind == "ExternalInput":
                if name != part_name:
                    in_names.append(name)
            elif alloc.kind == "ExternalOutput":
                out_names.append(name)
                shape = tuple(alloc.tensor_shape)
                dtype = mybir.dt.np(alloc.dtype)
                out_avals.append(jax.core.ShapedArray(shape, dtype))
                zero_outs.append(np.zeros(shape, dtype))
        self.in_names = list(in_names)
        self.out_names = out_names
        self.zero_outs = zero_outs
        n_params = len(in_names)
        n_outs = len(out_names)
        all_in_names = in_names + out_names
        if part_name is not None:
            all_in_names = all_in_names + [part_name]
        donate = tuple(range(n_params, n_params + n_outs))

        def _body_fn(*args):
            operands = list(args)
            if part_name is not None:
                operands.append(b2j.partition_id_tensor())
            outs = b2j._bass_exec_p.bind(
                *operands,
                out_avals=tuple(out_avals),
                in_names=tuple(all_in_names),
                out_names=tuple(out_names),
                lowering_input_output_aliases=(),
                sim_require_finite=True,
                sim_require_nnan=True,
                nc=nc,
            )
            return tuple(outs)

        devices = jax.devices()[:NCORES]
        mesh = Mesh(np.asarray(devices), ("core",))
        self.fn = jax.jit(
            shard_map(
                _body_fn, mesh=mesh,
                in_specs=(PartitionSpec("core"),) * (n_params + n_outs),
                out_specs=(PartitionSpec("core"),) * n_outs,
                check_rep=False,
            ),
            donate_argnums=donate,
            keep_unused=True,
        )

    def __call__(self, in_maps):
        concat_in = [
            np.concatenate([np.asarray(m[name]) for m in in_maps], axis=0)
            for name in self.in_names
        ]
        concat_zeros = [
            np.zeros((NCORES * z.shape[0], *z.shape[1:]), z.dtype)
            for z in self.zero_outs
        ]
        out_arrs = self.fn(*concat_in, *concat_zeros)
        return [
            {
                name: np.asarray(out_arrs[i]).reshape(NCORES, -1)[c]
                for i, name in enumerate(self.out_names)
            }
            for c in range(NCORES)
        ]


_runner_cache = {}


def _get_runner(reps: int = 1):
    r = _runner_cache.get(reps)
    if r is None:
        r = _runner_cache[reps] = _Runner(_get_nc(reps))
    return r


def _wrap_idx(v: np.ndarray) -> np.ndarray:
    """[J] -> [128, 2*JG/16] int16: group g = j // JG streams the pairs
    (v, v+1024) for its outputs, wrapped at [16*g + t%16, t//16]."""
    v = v.astype(np.int16)
    v2 = np.empty(2 * J, np.int16)
    v2[0::2] = v
    v2[1::2] = v + 1024
    w = v2.reshape(8, 2 * JG // 16, 16)   # [g, col, p16]
    w = w.transpose(0, 2, 1).reshape(128, 2 * JG // 16)
    return np.ascontiguousarray(w)


def prepare_in_maps(hidden_states, W_seq, W_hid, all_indices):
    x_bf = [np.ascontiguousarray(hidden_states[b].astype(ml_dtypes.bfloat16))
            for b in range(B)]
    ws_t = np.ascontiguousarray(W_seq.T.astype(ml_dtypes.bfloat16))
    wh_t = np.ascontiguousarray(W_hid.T.astype(ml_dtypes.bfloat16))
    idx_pairs = []
    for q in range(4):
        seg = all_indices[q * J:(q + 1) * J]
        idx_pairs.append((_wrap_idx(seg[:, 0]), _wrap_idx(seg[:, 1])))
    in_maps = []
    for c in range(NCORES):
        b, q = c // 4, c % 4
        ind = np.zeros((128, 8), np.float32)
        for g in range(8):
            ind[16 * g:16 * (g + 1), g] = 1.0
        in_maps.append({
            "x": x_bf[b],
            "wseq_t": ws_t,
            "whid_t": wh_t,
            "idx_s": idx_pairs[q][0],
            "idx_h": idx_pairs[q][1],
            "ind": ind,
        })
    return in_maps


def kernel(hidden_states, W_seq, W_hid, all_indices):
    hidden_states = np.asarray(hidden_states)
    W_seq = np.asarray(W_seq)
    W_hid = np.asarray(W_hid)
    all_indices = np.asarray(all_indices)

    runner = _get_runner()
    in_maps = prepare_in_maps(hidden_states, W_seq, W_hid, all_indices)
    results = runner(in_maps)

    out = np.empty((B, N), dtype=np.float32)
    for c in range(NCORES):
        b, q = c // 4, c % 4
        o = results[c]["out"].reshape(8, JG)
        # out[g, jj] holds output j = g*JG + jj of this core's quarter
        out[b, q * J:(q + 1) * J] = o.reshape(J)
    return out.reshape(B, S, H)


# revision 27
# speedup vs baseline: 187.5262x; 1.6230x over previous
"""Trainium2 Bass kernel for nn_CPCircuitLayer (embedding_lookup).

Math: A = X @ W_seq^T  [S,R];  Bm = X^T @ W_hid^T  [H,R]
      out[b, n] = dot(A[b, idx_s[n]], Bm[b, idx_h[n]]),  out -> [B, S, H]

Sharding (8 cores, no collectives): core c handles batch b = c//4 and the
quarter q = c%4 of the N = S*H index list (J = N/4 indices). Both factor
tables are computed redundantly per batch group from the full X[b].

Per-core device pipeline:
  1. Load X[b] (bf16) + transposed copy via HWDGE transpose-DMA.
  2. PE matmuls (bf16 in, f32 psum): A^T and Bm^T [32, 1024].
  3. Repack to per-lane split-R tables: partition p holds columns
     2*(p%16), 2*(p%16)+1 of the factor interleaved ([128, 1024, 2] f32),
     via a DRAM bounce + 8x partition-group broadcast load.
  4. ap_gather (GPSIMD FIFO): each 16-partition group streams its own
     indices; one instruction gathers NIdx rows x 8 groups.
  5. DVE mul + pair-sum, then PE block-indicator matmul reduces the 16
     lanes x 2 of each group -> psum [8, n] -> out.
"""

import numpy as np
import ml_dtypes
from contextlib import ExitStack

import concourse.bass as bass
import concourse.mybir as mybir
import concourse.tile as tile
from concourse import bacc

B, S, H, R = 2, 1024, 1024, 32
N = S * H
NCORES = 8
J = N // 4            # outputs per core (one batch, quarter of N) = 262144
JG = J // 8           # outputs per 16-partition group = 32768
NIdx = 2048           # indices per group per ap_gather instruction
RNDS = JG // NIdx     # 16 gather rounds per table
GRP_D = 2             # table f32 per lane (R = 16 lanes * 2)
SKIP_GATHER = False   # timing experiment: drop ap_gather instructions

F32 = mybir.dt.float32
BF16 = mybir.dt.bfloat16
I16 = mybir.dt.int16


def _build(reps: int = 1):
    nc = bacc.Bacc()
    x = nc.declare_dram_parameter("x", [S, H], BF16, False)
    wseq_t = nc.declare_dram_parameter("wseq_t", [H, R], BF16, False)
    whid_t = nc.declare_dram_parameter("whid_t", [S, R], BF16, False)
    # per-group index streams, wrapped: group g's jj-th index lives at
    # [16*g + jj%16, jj//16]
    idx_s = nc.declare_dram_parameter("idx_s", [128, 2 * JG // 16], I16, False)
    idx_h = nc.declare_dram_parameter("idx_h", [128, 2 * JG // 16], I16, False)
    ind_in = nc.declare_dram_parameter("ind", [128, 8], F32, False)
    out = nc.declare_dram_parameter("out", [8, JG], F32, True)
    ta_dram = nc.dram_tensor("ta", [R, S], F32)   # A^T bounce
    tb_dram = nc.dram_tensor("tb", [R, H], F32)   # Bm^T bounce

    with tile.TileContext(nc) as tc, ExitStack() as ctx:
        base = ctx.enter_context(tc.tile_pool(name="base", bufs=1))
        psum = ctx.enter_context(tc.tile_pool(name="psum", bufs=2, space="PSUM"))
        rpsum = ctx.enter_context(tc.tile_pool(name="rpsum", bufs=1, space="PSUM"))
        stage = ctx.enter_context(tc.tile_pool(name="stage", bufs=2))
        gap = ctx.enter_context(tc.tile_pool(name="gap", bufs=2))
        gbp = ctx.enter_context(tc.tile_pool(name="gbp", bufs=2))
        prodp = ctx.enter_context(tc.tile_pool(name="prodp", bufs=2))
        otp = ctx.enter_context(tc.tile_pool(name="otp", bufs=1))

        # --- loads -------------------------------------------------------
        x_sb = base.tile([128, 8, H], BF16)       # X[s,h]: p=s%128, k=s//128
        xt_sb = base.tile([128, 8, S], BF16)      # X^T[h,s]: p=h%128, k=h//128
        ws_sb = base.tile([128, 8, R], BF16)      # W_seq^T rows (h-major)
        wh_sb = base.tile([128, 8, R], BF16)      # W_hid^T rows (s-major)
        isb_s = base.tile([128, 2 * JG // 16], I16)
        isb_h = base.tile([128, 2 * JG // 16], I16)
        ind_sb = base.tile([128, 8], F32)         # block indicator for reduce
        ta_sb = base.tile([128, 2 * S], F32)
        tb_sb = base.tile([128, 2 * H], F32)

        nc.sync.dma_start(
            out=x_sb[:],
            in_=bass.AP(tensor=x[:].tensor, offset=0,
                        ap=[[H, 128], [128 * H, 8], [1, H]]),
        )
        for k in range(8):
            nc.sync.dma_start_transpose(
                out=xt_sb[:, k, :], in_=x[:, 128 * k:128 * (k + 1)]
            )
        nc.sync.dma_start(
            out=ws_sb[:],
            in_=bass.AP(tensor=wseq_t[:].tensor, offset=0,
                        ap=[[R, 128], [128 * R, 8], [1, R]]),
        )
        nc.sync.dma_start(
            out=wh_sb[:],
            in_=bass.AP(tensor=whid_t[:].tensor, offset=0,
                        ap=[[R, 128], [128 * R, 8], [1, R]]),
        )
        nc.sync.dma_start(out=isb_s[:], in_=idx_s[:])
        nc.sync.dma_start(out=isb_h[:], in_=idx_h[:])

        nc.sync.dma_start(out=ind_sb[:], in_=ind_in[:])

        for _ in range(reps):
            _body(nc, psum, rpsum, stage, gap, gbp, prodp, otp,
                  x_sb, xt_sb, ws_sb, wh_sb, isb_s, isb_h, ind_sb,
                  ta_sb, tb_sb, ta_dram, tb_dram, out)
    nc.compile()
    return nc


def _body(nc, psum, rpsum, stage, gap, gbp, prodp, otp,
          x_sb, xt_sb, ws_sb, wh_sb, isb_s, isb_h, ind_sb,
          ta_sb, tb_sb, ta_dram, tb_dram, out):
    # --- factor transposes on PE: F^T [32, 1024] ------------------------
    # A^T[r, s] = sum_h Wseq^T[h, r] X^T[h, s]; Bm^T[r, h] = sum_s ...
    for (tdram, lhs_w, rhs_x) in ((ta_dram, ws_sb, xt_sb),
                                  (tb_dram, wh_sb, x_sb)):
        ft = stage.tile([R, S], F32, tag="ft")
        for nh in range(2):
            pt = psum.tile([R, S // 2], F32, tag="pt")
            for k in range(8):
                nc.tensor.matmul(
                    out=pt[:],
                    lhsT=lhs_w[:, k, :],
                    rhs=rhs_x[:, k, nh * 512:(nh + 1) * 512],
                    start=(k == 0), stop=(k == 7),
                )
            nc.vector.tensor_copy(out=ft[:, nh * 512:(nh + 1) * 512], in_=pt[:])
        nc.gpsimd.dma_start(out=tdram[:], in_=ft[:])

    # broadcast tables back, lane-split d=2 interleaved: partition p
    # (lane l = p%16) holds tab[p, v, d] = F^T[2l+d, v]
    for (tdram, tsb, V) in ((ta_dram, ta_sb, S), (tb_dram, tb_sb, H)):
        nc.gpsimd.dma_start(
            out=tsb[:],
            in_=bass.AP(tensor=tdram[:].tensor, offset=0,
                        ap=[[0, 8], [2 * V, 16], [1, 2 * V]]),
        )

    # --- gather + reduce ------------------------------------------------
    ot = None
    for rnd in range(RNDS):
        isl = slice(rnd * (2 * NIdx // 16), (rnd + 1) * (2 * NIdx // 16))
        ga = gap.tile([128, NIdx, GRP_D], F32, tag="ga")
        gb = gbp.tile([128, NIdx, GRP_D], F32, tag="gb")
        ga_flat = bass.AP(tensor=ga[:].tensor, offset=ga[:].offset,
                          ap=[list(ga[:].ap[0]), [1, 2 * NIdx], [1, 1]])
        gb_flat = bass.AP(tensor=gb[:].tensor, offset=gb[:].offset,
                          ap=[list(gb[:].ap[0]), [1, 2 * NIdx], [1, 1]])
        if SKIP_GATHER:
            nc.vector.memset(ga[:], 0.0)
            nc.vector.memset(gb[:], 0.0)
        else:
            nc.gpsimd.ap_gather(
                out_ap=ga_flat, in_ap=ta_sb[:], idxs_ap=isb_s[:, isl],
                channels=128, num_elems=2 * S, d=1, num_idxs=2 * NIdx,
            )
            nc.gpsimd.ap_gather(
                out_ap=gb_flat, in_ap=tb_sb[:], idxs_ap=isb_h[:, isl],
                channels=128, num_elems=2 * H, d=1, num_idxs=2 * NIdx,
            )
        prod = prodp.tile([128, NIdx, GRP_D], F32, tag="prod")
        nc.vector.tensor_mul(prod[:], ga[:], gb[:])
        p2 = prodp.tile([128, NIdx], F32, tag="p2")
        nc.vector.tensor_add(p2[:], prod[:, :, 0], prod[:, :, 1])
        # reduce 16 lanes per group via block-indicator matmul; all four
        # 512-col results land in one 4-bank psum tile -> single copy;
        # out-DMA once per two rounds
        if rnd % 2 == 0:
            ot = otp.tile([8, 2 * NIdx], F32, tag="ot")
        rp4 = rpsum.tile([8, NIdx], F32, tag="rp4")
        for t in range(NIdx // 512):
            nc.tensor.matmul(
                out=rp4[:, t * 512:(t + 1) * 512],
                lhsT=ind_sb[:],
                rhs=p2[:, t * 512:(t + 1) * 512],
                start=True, stop=True,
            )
        half = (rnd % 2) * NIdx
        nc.scalar.copy(out=ot[:, half:half + NIdx], in_=rp4[:])
        if rnd % 2 == 1:
            nc.sync.dma_start(
                out=bass.AP(tensor=out[:].tensor, offset=(rnd - 1) * NIdx,
                            ap=[[JG, 8], [1, 2 * NIdx]]),
                in_=ot[:],
            )


_nc_cache_by_reps = {}


def _get_nc(reps: int = 1):
    nc = _nc_cache_by_reps.get(reps)
    if nc is None:
        nc = _nc_cache_by_reps[reps] = _build(reps)
    return nc


class _Runner:
    """Trace/compile the SPMD executable once; reuse across calls."""

    def __init__(self, nc):
        import jax
        from jax.experimental.shard_map import shard_map
        from jax.sharding import Mesh, PartitionSpec
        import concourse.bass2jax as b2j

        b2j.install_neuronx_cc_hook()
        self.nc = nc
        part_name = (nc.partition_id_tensor.name
                     if nc.partition_id_tensor else None)
        in_names, out_names, out_avals = [], [], []
        zero_outs = []
        for alloc in nc.m.functions[0].allocations:
            if not isinstance(alloc, mybir.MemoryLocationSet):
                continue
            name = alloc.memorylocations[0].name
            if alloc.kind == "ExternalInput":
                if name != part_name:
                    in_names.append(name)
            elif alloc.kind == "ExternalOutput":
                out_names.append(name)
                shape = tuple(alloc.tensor_shape)
                dtype = mybir.dt.np(alloc.dtype)
                out_avals.append(jax.core.ShapedArray(shape, dtype))
                zero_outs.append(np.zeros(shape, dtype))
        self.in_names = list(in_names)
        self.out_names = out_names
        self.zero_outs = zero_outs
        n_params = len(in_names)
        n_outs = len(out_names)
        all_in_names = in_names + out_names
        if part_name is not None:
            all_in_names = all_in_names + [part_name]
        donate = tuple(range(n_params, n_params + n_outs))

        def _body_fn(*args):
            operands = list(args)
            if part_name is not None:
                operands.append(b2j.partition_id_tensor())
            outs = b2j._bass_exec_p.bind(
                *operands,
                out_avals=tuple(out_avals),
                in_names=tuple(all_in_names),
                out_names=tuple(out_names),
                lowering_input_output_aliases=(),
                sim_require_finite=True,
                sim_require_nnan=True,
                nc=nc,
            )
            return tuple(outs)

        devices = jax.devices()[:NCORES]
        mesh = Mesh(np.asarray(devices), ("core",))
        self.fn = jax.jit(
            shard_map(
                _body_fn, mesh=mesh,
                in_specs=(PartitionSpec("core"),) * (n_params + n_outs),
                out_specs=(PartitionSpec("core"),) * n_outs,
                check_rep=False,
            ),
            donate_argnums=donate,
            keep_unused=True,
        )

    def __call__(self, in_maps):
        concat_in = [
            np.concatenate([np.asarray(m[name]) for m in in_maps], axis=0)
            for name in self.in_names
        ]
        concat_zeros = [
            np.zeros((NCORES * z.shape[0], *z.shape[1:]), z.dtype)
            for z in self.zero_outs
        ]
        out_arrs = self.fn(*concat_in, *concat_zeros)
        return [
            {
                name: np.asarray(out_arrs[i]).reshape(NCORES, -1)[c]
                for i, name in enumerate(self.out_names)
            }
            for c in range(NCORES)
        ]


_runner_cache = {}


def _get_runner(reps: int = 1):
    r = _runner_cache.get(reps)
    if r is None:
        r = _runner_cache[reps] = _Runner(_get_nc(reps))
    return r


def _wrap_idx(v: np.ndarray) -> np.ndarray:
    """[J] -> [128, 2*JG/16] int16: group g = j // JG streams the pairs
    (v, v+1024) for its outputs, wrapped at [16*g + t%16, t//16]."""
    v = v.astype(np.int16)
    v2 = np.empty(2 * J, np.int16)
    v2[0::2] = v
    v2[1::2] = v + 1024
    w = v2.reshape(8, 2 * JG // 16, 16)   # [g, col, p16]
    w = w.transpose(0, 2, 1).reshape(128, 2 * JG // 16)
    return np.ascontiguousarray(w)


def prepare_in_maps(hidden_states, W_seq, W_hid, all_indices):
    x_bf = [np.ascontiguousarray(hidden_states[b].astype(ml_dtypes.bfloat16))
            for b in range(B)]
    ws_t = np.ascontiguousarray(W_seq.T.astype(ml_dtypes.bfloat16))
    wh_t = np.ascontiguousarray(W_hid.T.astype(ml_dtypes.bfloat16))
    idx_pairs = []
    for q in range(4):
        seg = all_indices[q * J:(q + 1) * J]
        idx_pairs.append((_wrap_idx(seg[:, 0]), _wrap_idx(seg[:, 1])))
    in_maps = []
    for c in range(NCORES):
        b, q = c // 4, c % 4
        ind = np.zeros((128, 8), np.float32)
        for g in range(8):
            ind[16 * g:16 * (g + 1), g] = 1.0
        in_maps.append({
            "x": x_bf[b],
            "wseq_t": ws_t,
            "whid_t": wh_t,
            "idx_s": idx_pairs[q][0],
            "idx_h": idx_pairs[q][1],
            "ind": ind,
        })
    return in_maps


def kernel(hidden_states, W_seq, W_hid, all_indices):
    hidden_states = np.asarray(hidden_states)
    W_seq = np.asarray(W_seq)
    W_hid = np.asarray(W_hid)
    all_indices = np.asarray(all_indices)

    runner = _get_runner()
    in_maps = prepare_in_maps(hidden_states, W_seq, W_hid, all_indices)
    results = runner(in_maps)

    out = np.empty((B, N), dtype=np.float32)
    for c in range(NCORES):
        b, q = c // 4, c % 4
        o = results[c]["out"].reshape(8, JG)
        # out[g, jj] holds output j = g*JG + jj of this core's quarter
        out[b, q * J:(q + 1) * J] = o.reshape(J)
    return out.reshape(B, S, H)
